# revision 4
# baseline (speedup 1.0000x reference)
"""ChebyKAN layer on 8 Trainium2 NeuronCores.

y[b,o] = sum_{i,d} T_d(tanh(x[b,i])) * coeffs[i,o,d],  d=0..8

The Chebyshev basis is re-parameterized (exact 9x9 linear transform of the
coefficients on host) into products of Chebyshev values the device builds
with ACT squares + DVE fused ops:
  G0=1, G1=t, G2=t^2, G3q=(G2-3/4)t=T3/4, G4=(2G2-1)^2=T2^2,
  G5q=(G4-1/2)t=(T5+T3)/4, G6=(4G3q)^2=T3^2, G7q=(G6-1/2)t=(T7+T5)/4,
  G8=(2G4-1)^2=T4^2

v2 vs v1 (fp32r, 613us):
- The G0==1 level contributed a per-output constant bias[o] = sum_i C'[i,o,0];
  it is folded into the PSUM eviction as a broadcast bias add. This removes
  1/9 of all matmul work (2304 -> 2048 matmuls per core).
- Matmul operands are bf16: the G chain is computed in fp32 on ACT/DVE and
  rounded to bf16 once per operand (max rel err ~3e-3 vs the 2e-2 budget).
  bf16 weights use the PE's fast-weight-load path, hiding LDWEIGHTS under the
  512-column matmul stream; fp32r weights loaded 4B-wide and could not hide
  (~53ns/MM penalty = the v1 gap to roofline).
- MACRO=512 so all (output-half x batch-subtile) groups fit the 8 PSUM banks
  simultaneously: each G tile is built once and consumed by all 8 groups.

Sharding: data-parallel over batch (2048 rows/core), coeffs replicated.
"""

import numpy as np
import concourse.mybir as mybir
import concourse.tile as tile
from concourse import bacc
from concourse.bass_utils import run_bass_kernel_spmd

B, I, O, D1 = 16384, 1024, 1024, 9
CORES = 8
BC = B // CORES            # 2048 batch rows per core
P = 128
MACRO = 512                # batch rows per psum generation
NM = BC // MACRO           # 4
BT = MACRO // P            # 4 batch subtiles
OH = 512                   # output columns per psum bank
NOH = O // OH              # 2
IB = I // P                # 8 i-blocks
LV = 8                     # matmul levels G1..G8 (G0 folded into bias)

F32 = mybir.dt.float32
BF16 = mybir.dt.bfloat16
AF = mybir.ActivationFunctionType
OP = mybir.AluOpType

_CACHE = {}
_last_in_maps = None

# G_k = sum_d M[k,d] T_d  (exact); host solves M^T C' = C
_M = np.zeros((9, 9))
_M[0, 0] = 1; _M[1, 1] = 1
_M[2, 0] = .5; _M[2, 2] = .5
_M[3, 3] = .25
_M[4, 0] = .5; _M[4, 4] = .5
_M[5, 3] = .25; _M[5, 5] = .25
_M[6, 0] = .5; _M[6, 6] = .5
_M[7, 5] = .25; _M[7, 7] = .25
_M[8, 0] = .5; _M[8, 8] = .5
_A = np.linalg.inv(_M.T)


def _emit_body(nc, xt_d, c2_d, y_d, pools, neg1, bias_t):
    xp, mp, bp, cp, op_, pp = pools
    for m in range(NM):
        psum = {}
        for oh in range(NOH):
            for bt in range(BT):
                psum[(oh, bt)] = pp.tile(
                    [P, OH], F32, tag=f"ps{oh * BT + bt}", name=f"ps_{m}_{oh}_{bt}"
                )
        for ib in range(IB):
            sfx = f"{m}_{ib}"
            xt = xp.tile([P, MACRO], F32, tag="xt", name=f"xt_{sfx}")
            nc.sync.dma_start(
                xt[:], xt_d[ib * P:(ib + 1) * P, m * MACRO:(m + 1) * MACRO]
            )
            # fp32 masters (ACT/DVE), one bf16 rounding per operand
            t = mp.tile([P, MACRO], F32, tag="t", name=f"t_{sfx}")
            nc.scalar.activation(t[:], xt[:], AF.Tanh)
            g2 = mp.tile([P, MACRO], F32, tag="g2", name=f"g2_{sfx}")
            nc.scalar.activation(g2[:], t[:], AF.Square)
            g3 = mp.tile([P, MACRO], F32, tag="g3", name=f"g3_{sfx}")
            nc.vector.scalar_tensor_tensor(g3[:], g2[:], 0.75, t[:], OP.subtract, OP.mult)
            g4 = mp.tile([P, MACRO], F32, tag="g4", name=f"g4_{sfx}")
            nc.scalar.activation(g4[:], g2[:], AF.Square, bias=neg1[:], scale=2.0)
            g6 = mp.tile([P, MACRO], F32, tag="g6", name=f"g6_{sfx}")
            nc.scalar.activation(g6[:], g3[:], AF.Square, scale=4.0)

            gb = {}
            for lvl, src in ((1, t), (2, g2), (4, g4)):
                w = bp.tile([P, MACRO], BF16, tag=f"b{lvl}", name=f"b{lvl}_{sfx}")
                nc.scalar.activation(w[:], src[:], AF.Copy)
                gb[lvl] = w
            for lvl, src in ((3, g3), (6, g6)):
                w = bp.tile([P, MACRO], BF16, tag=f"b{lvl}", name=f"b{lvl}_{sfx}")
                nc.vector.tensor_copy(w[:], src[:])
                gb[lvl] = w
            w5 = bp.tile([P, MACRO], BF16, tag="b5", name=f"b5_{sfx}")
            nc.vector.scalar_tensor_tensor(w5[:], g4[:], 0.5, t[:], OP.subtract, OP.mult)
            gb[5] = w5
            w7 = bp.tile([P, MACRO], BF16, tag="b7", name=f"b7_{sfx}")
            nc.vector.scalar_tensor_tensor(w7[:], g6[:], 0.5, t[:], OP.subtract, OP.mult)
            gb[7] = w7
            w8 = bp.tile([P, MACRO], BF16, tag="b8", name=f"b8_{sfx}")
            nc.scalar.activation(w8[:], g4[:], AF.Square, bias=neg1[:], scale=2.0)
            gb[8] = w8

            for lvl in range(1, LV + 1):
                c2t = cp.tile([P, O], BF16, tag="c2", name=f"c2_{sfx}_{lvl}")
                r0 = (lvl - 1) * I + ib * P
                nc.sync.dma_start(c2t[:], c2_d[r0:r0 + P, :])
                for bt in range(BT):
                    lhs = gb[lvl][:, bt * P:(bt + 1) * P]
                    for oh in range(NOH):
                        nc.tensor.matmul(
                            psum[(oh, bt)][:],
                            lhs,
                            c2t[:, oh * OH:(oh + 1) * OH],
                            start=(ib == 0 and lvl == 1),
                            stop=(ib == IB - 1 and lvl == LV),
                        )

        for oh in range(NOH):
            for bt in range(BT):
                ob = op_.tile([P, OH], F32, tag="ob", name=f"ob_{m}_{oh}_{bt}")
                nc.vector.scalar_tensor_tensor(
                    ob[:], psum[(oh, bt)][:], 1.0, bias_t[oh][:], OP.mult, OP.add
                )
                nc.sync.dma_start(
                    y_d[
                        m * MACRO + bt * P:m * MACRO + (bt + 1) * P,
                        oh * OH:(oh + 1) * OH,
                    ],
                    ob[:],
                )


def build_nc(loop_reps: int = 0):
    """loop_reps=0: plain kernel. loop_reps=N>0: body wrapped in a hardware
    For_i loop executing N times (same instruction footprint) — used by the
    timing harness to amplify device time over dispatch jitter."""
    nc = bacc.Bacc("TRN2", target_bir_lowering=False, debug=False, num_devices=CORES)
    xt_d = nc.dram_tensor("xt", [I, BC], F32, kind="ExternalInput")
    c2_d = nc.dram_tensor("c2", [LV * I, O], BF16, kind="ExternalInput")
    bias_d = nc.dram_tensor("bias", [P, O], F32, kind="ExternalInput")
    y_d = nc.dram_tensor("y", [BC, O], F32, kind="ExternalOutput")

    with tile.TileContext(nc) as tc:
        with (
            tc.tile_pool(name="xp", bufs=2) as xp,       # x staging
            tc.tile_pool(name="kp", bufs=1) as kp,       # constants
            tc.tile_pool(name="mp", bufs=2) as mp,       # fp32 G masters
            tc.tile_pool(name="bp", bufs=2) as bp,       # bf16 matmul weights
            tc.tile_pool(name="cp", bufs=4) as cp,       # coeff stream
            tc.tile_pool(name="op", bufs=2) as op_,      # psum eviction staging
            tc.tile_pool(name="pp", bufs=1, space="PSUM") as pp,
        ):
            neg1 = kp.tile([P, 1], F32, tag="neg1")
            nc.vector.memset(neg1[:], -1.0)
            bias_t = []
            for oh in range(NOH):
                bt_ = kp.tile([P, OH], F32, tag=f"bias{oh}")
                nc.sync.dma_start(bt_[:], bias_d[:, oh * OH:(oh + 1) * OH])
                bias_t.append(bt_)

            pools = (xp, mp, bp, cp, op_, pp)
            if loop_reps:
                with tc.For_i(0, loop_reps, 1):
                    _emit_body(nc, xt_d, c2_d, y_d, pools, neg1, bias_t)
            else:
                _emit_body(nc, xt_d, c2_d, y_d, pools, neg1, bias_t)
    nc.compile()
    return nc


def kernel(x: np.ndarray, cheby_coeffs: np.ndarray) -> np.ndarray:
    assert x.shape == (B, I) and cheby_coeffs.shape == (I, O, D1)
    if "nc" not in _CACHE:
        _CACHE["nc"] = build_nc()
    nc = _CACHE["nc"]

    xt = np.ascontiguousarray(x.T.astype(np.float32, copy=False))          # (I, B)
    cp = np.einsum("ed,iod->ioe", _A, cheby_coeffs.astype(np.float64))     # C' (I,O,9)
    bias = cp[:, :, 0].sum(axis=0).astype(np.float32)                      # (O,)
    bias_rep = np.ascontiguousarray(np.broadcast_to(bias, (P, O)))
    c2 = np.ascontiguousarray(
        np.transpose(cp[:, :, 1:], (2, 0, 1)).reshape(LV * I, O)
    ).astype(mybir.dt.np(BF16))

    in_maps = [
        {
            "xt": np.ascontiguousarray(xt[:, c * BC:(c + 1) * BC]),
            "c2": c2,
            "bias": bias_rep,
        }
        for c in range(CORES)
    ]
    global _last_in_maps
    _last_in_maps = in_maps
    res = run_bass_kernel_spmd(nc, in_maps, core_ids=list(range(CORES)))
    return np.concatenate([res.results[c]["y"] for c in range(CORES)], axis=0)


# revision 7
# speedup vs baseline: 3.0354x; 3.0354x over previous
"""ChebyKAN layer on 8 Trainium2 NeuronCores.

y[b,o] = sum_{i,d} T_d(tanh(x[b,i])) * coeffs[i,o,d],  d=0..8

The Chebyshev basis is re-parameterized (exact 9x9 linear transform of the
coefficients on host) into products of Chebyshev values the device builds
with ACT squares + DVE fused ops:
  G0=1, G1=t, G2=t^2, G3q=(G2-3/4)t=T3/4, G4=(2G2-1)^2=T2^2,
  G5q=(G4-1/2)t=(T5+T3)/4, G6=(4G3q)^2=T3^2, G7q=(G6-1/2)t=(T7+T5)/4,
  G8=(2G4-1)^2=T4^2

v2 vs v1 (fp32r, 613us):
- The G0==1 level contributed a per-output constant bias[o] = sum_i C'[i,o,0];
  it is folded into the PSUM eviction as a broadcast bias add. This removes
  1/9 of all matmul work (2304 -> 2048 matmuls per core).
- Matmul operands are bf16: the G chain is computed in fp32 on ACT/DVE and
  rounded to bf16 once per operand (max rel err ~3e-3 vs the 2e-2 budget).
  bf16 weights use the PE's fast-weight-load path, hiding LDWEIGHTS under the
  512-column matmul stream; fp32r weights loaded 4B-wide and could not hide
  (~53ns/MM penalty = the v1 gap to roofline).
- MACRO=512 so all (output-half x batch-subtile) groups fit the 8 PSUM banks
  simultaneously: each G tile is built once and consumed by all 8 groups.

Sharding: data-parallel over batch (2048 rows/core), coeffs replicated.
"""

import numpy as np
import concourse.mybir as mybir
import concourse.tile as tile
from concourse import bacc
from concourse.bass_utils import run_bass_kernel_spmd

B, I, O, D1 = 16384, 1024, 1024, 9
CORES = 8
BC = B // CORES            # 2048 batch rows per core
P = 128
MACRO = 512                # batch rows per psum generation
NM = BC // MACRO           # 4
BT = MACRO // P            # 4 batch subtiles
OH = 512                   # output columns per psum bank
NOH = O // OH              # 2
IB = I // P                # 8 i-blocks
LV = 8                     # matmul levels G1..G8 (G0 folded into bias)

F32 = mybir.dt.float32
BF16 = mybir.dt.bfloat16
AF = mybir.ActivationFunctionType
OP = mybir.AluOpType

_CACHE = {}
_last_in_maps = None

# G_k = sum_d M[k,d] T_d  (exact); host solves M^T C' = C
_M = np.zeros((9, 9))
_M[0, 0] = 1; _M[1, 1] = 1
_M[2, 0] = .5; _M[2, 2] = .5
_M[3, 3] = .25
_M[4, 0] = .5; _M[4, 4] = .5
_M[5, 3] = .25; _M[5, 5] = .25
_M[6, 0] = .5; _M[6, 6] = .5
_M[7, 5] = .25; _M[7, 7] = .25
_M[8, 0] = .5; _M[8, 8] = .5
_A = np.linalg.inv(_M.T)


def _emit_body(nc, xt_d, c2_d, y_d, pools, neg1, bias_t):
    xp, mp, bp, cp, op_, pp = pools
    for m in range(NM):
        psum = {}
        for oh in range(NOH):
            for bt in range(BT):
                psum[(oh, bt)] = pp.tile(
                    [P, OH], F32, tag=f"ps{oh * BT + bt}", name=f"ps_{m}_{oh}_{bt}"
                )
        for ib in range(IB):
            sfx = f"{m}_{ib}"
            xt = xp.tile([P, MACRO], F32, tag="xt", name=f"xt_{sfx}")
            # ACT-issued (own HWDGE queue): keeps the SP queue exclusively
            # for the latency-critical c2 stream
            nc.scalar.dma_start(
                xt[:], xt_d[ib * P:(ib + 1) * P, m * MACRO:(m + 1) * MACRO]
            )
            # fp32 masters (ACT/DVE), one bf16 rounding per operand
            t = mp.tile([P, MACRO], F32, tag="t", name=f"t_{sfx}")
            nc.scalar.activation(t[:], xt[:], AF.Tanh)
            g2 = mp.tile([P, MACRO], F32, tag="g2", name=f"g2_{sfx}")
            nc.scalar.activation(g2[:], t[:], AF.Square)
            g3 = mp.tile([P, MACRO], F32, tag="g3", name=f"g3_{sfx}")
            nc.vector.scalar_tensor_tensor(g3[:], g2[:], 0.75, t[:], OP.subtract, OP.mult)
            g4 = mp.tile([P, MACRO], F32, tag="g4", name=f"g4_{sfx}")
            nc.scalar.activation(g4[:], g2[:], AF.Square, bias=neg1[:], scale=2.0)
            g6 = mp.tile([P, MACRO], F32, tag="g6", name=f"g6_{sfx}")
            nc.scalar.activation(g6[:], g3[:], AF.Square, scale=4.0)

            gb = {}
            for lvl, src in ((1, t), (2, g2), (4, g4)):
                w = bp.tile([P, MACRO], BF16, tag=f"b{lvl}", name=f"b{lvl}_{sfx}")
                nc.scalar.activation(w[:], src[:], AF.Copy)
                gb[lvl] = w
            for lvl, src in ((3, g3), (6, g6)):
                w = bp.tile([P, MACRO], BF16, tag=f"b{lvl}", name=f"b{lvl}_{sfx}")
                nc.vector.tensor_copy(w[:], src[:])
                gb[lvl] = w
            w5 = bp.tile([P, MACRO], BF16, tag="b5", name=f"b5_{sfx}")
            nc.vector.scalar_tensor_tensor(w5[:], g4[:], 0.5, t[:], OP.subtract, OP.mult)
            gb[5] = w5
            w7 = bp.tile([P, MACRO], BF16, tag="b7", name=f"b7_{sfx}")
            nc.vector.scalar_tensor_tensor(w7[:], g6[:], 0.5, t[:], OP.subtract, OP.mult)
            gb[7] = w7
            w8 = bp.tile([P, MACRO], BF16, tag="b8", name=f"b8_{sfx}")
            nc.scalar.activation(w8[:], g4[:], AF.Square, bias=neg1[:], scale=2.0)
            gb[8] = w8

            for lvl in range(1, LV + 1):
                c2t = cp.tile([P, O], BF16, tag="c2", name=f"c2_{sfx}_{lvl}")
                r0 = (lvl - 1) * I + ib * P
                nc.sync.dma_start(c2t[:], c2_d[r0:r0 + P, :])
                for bt in range(BT):
                    lhs = gb[lvl][:, bt * P:(bt + 1) * P]
                    for oh in range(NOH):
                        nc.tensor.matmul(
                            psum[(oh, bt)][:],
                            lhs,
                            c2t[:, oh * OH:(oh + 1) * OH],
                            start=(ib == 0 and lvl == 1),
                            stop=(ib == IB - 1 and lvl == LV),
                        )

        for oh in range(NOH):
            for bt in range(BT):
                ob = op_.tile([P, OH], F32, tag="ob", name=f"ob_{m}_{oh}_{bt}")
                nc.vector.scalar_tensor_tensor(
                    ob[:], psum[(oh, bt)][:], 1.0, bias_t[oh][:], OP.mult, OP.add
                )
                # GpSimd/SWDGE queue: y write-back never blocks input prefetch
                nc.gpsimd.dma_start(
                    y_d[
                        m * MACRO + bt * P:m * MACRO + (bt + 1) * P,
                        oh * OH:(oh + 1) * OH,
                    ],
                    ob[:],
                )


def build_nc(loop_reps: int = 0):
    """loop_reps=0: plain kernel. loop_reps=N>0: body wrapped in a hardware
    For_i loop executing N times (same instruction footprint) — used by the
    timing harness to amplify device time over dispatch jitter."""
    nc = bacc.Bacc("TRN2", target_bir_lowering=False, debug=False, num_devices=CORES)
    xt_d = nc.dram_tensor("xt", [I, BC], F32, kind="ExternalInput")
    c2_d = nc.dram_tensor("c2", [LV * I, O], BF16, kind="ExternalInput")
    bias_d = nc.dram_tensor("bias", [P, O], F32, kind="ExternalInput")
    y_d = nc.dram_tensor("y", [BC, O], F32, kind="ExternalOutput")

    with tile.TileContext(nc) as tc:
        with (
            tc.tile_pool(name="xp", bufs=3) as xp,       # x staging
            tc.tile_pool(name="kp", bufs=1) as kp,       # constants
            tc.tile_pool(name="mp", bufs=3) as mp,       # fp32 G masters
            tc.tile_pool(name="bp", bufs=3) as bp,       # bf16 matmul weights
            tc.tile_pool(name="cp", bufs=12) as cp,      # coeff stream (covers ~10us DMA latency)
            tc.tile_pool(name="op", bufs=2) as op_,      # psum eviction staging
            tc.tile_pool(name="pp", bufs=1, space="PSUM") as pp,
        ):
            neg1 = kp.tile([P, 1], F32, tag="neg1")
            nc.vector.memset(neg1[:], -1.0)
            bias_t = []
            for oh in range(NOH):
                bt_ = kp.tile([P, OH], F32, tag=f"bias{oh}")
                nc.sync.dma_start(bt_[:], bias_d[:, oh * OH:(oh + 1) * OH])
                bias_t.append(bt_)

            pools = (xp, mp, bp, cp, op_, pp)
            if loop_reps:
                with tc.For_i(0, loop_reps, 1):
                    _emit_body(nc, xt_d, c2_d, y_d, pools, neg1, bias_t)
            else:
                _emit_body(nc, xt_d, c2_d, y_d, pools, neg1, bias_t)
    nc.compile()
    return nc


def kernel(x: np.ndarray, cheby_coeffs: np.ndarray) -> np.ndarray:
    assert x.shape == (B, I) and cheby_coeffs.shape == (I, O, D1)
    if "nc" not in _CACHE:
        _CACHE["nc"] = build_nc()
    nc = _CACHE["nc"]

    xt = np.ascontiguousarray(x.T.astype(np.float32, copy=False))          # (I, B)
    cp = np.einsum("ed,iod->ioe", _A, cheby_coeffs.astype(np.float64))     # C' (I,O,9)
    bias = cp[:, :, 0].sum(axis=0).astype(np.float32)                      # (O,)
    bias_rep = np.ascontiguousarray(np.broadcast_to(bias, (P, O)))
    c2 = np.ascontiguousarray(
        np.transpose(cp[:, :, 1:], (2, 0, 1)).reshape(LV * I, O)
    ).astype(mybir.dt.np(BF16))

    in_maps = [
        {
            "xt": np.ascontiguousarray(xt[:, c * BC:(c + 1) * BC]),
            "c2": c2,
            "bias": bias_rep,
        }
        for c in range(CORES)
    ]
    global _last_in_maps
    _last_in_maps = in_maps
    res = run_bass_kernel_spmd(nc, in_maps, core_ids=list(range(CORES)))
    return np.concatenate([res.results[c]["y"] for c in range(CORES)], axis=0)
